# revision 38
# baseline (speedup 1.0000x reference)
"""Trainium2 Bass kernel for ErosionP4 (P4 group-equivariant grayscale erosion).

Reference computation (shapes hardcoded):
  x: [B=4, G=4, H=96, W=96, C=4] fp32, kernel: [5, 5, 3, C=4, F=8] fp32
  out[b,g,h,w,f] = sum_c min_{k,dy,dx} ( ygp[b,g,k,h+dy,w+dx,c] - krev[g,dy,dx,k,c,f] )
  where ygp[b,g,k] = x[b, (g+k-1) mod 4] spatially padded with +inf and
  krev = the 4 planar rotations of the depth-rotated SE, spatially reversed.

Sharding: core -> (g = core//2, f-half = core%2).  Each core computes all 4
batches for one group-rotation g and 4 of the 8 filters.

Packing "cp32": partition p = 32*c + j carries channel c = p//32 and row
offset j = p%32; chunk m (free-dim index) covers image row h = 32*m + j.
The SE value for a (tap, f) column then depends only on the partition, so
one [128,1] per-partition operand serves the full free width (3 chunks x 4
batches x 96 = 1152 elements), which lets the Scalar engine absorb subtract
work via its per-partition bias operand at full width.

Engine split per tap (3 depth x 5 dy x 5 dx = 75 taps, 4 filter columns):
  - DVE (vector): tensor_scalar subtract (4x fp16) + tensor_tensor min (2x)
  - ACT (scalar): Identity activation with bias = -kk absorbs a configurable
    fraction of the subtracts (KCFG_ACT16 / 16 of the columns)
  - GPSIMD (pool): whole taps (all 4 columns, ts+tt into its own private
    accumulator, merged at rep end) for KCFG_GPTAPS of the 75 taps
The channel sum happens on the host (c pieces are partition-misaligned).
"""

import os
from contextlib import ExitStack

import numpy as np

import concourse.bass as bass
import concourse.mybir as mybir
import concourse.tile as tile
from concourse.bass_utils import run_bass_kernel_spmd

B, G, H, W, C = 4, 4, 96, 96, 4
KH, KW, F = 5, 5, 8
PAD = 2
HP, WP = H + PAD * 2, W + PAD * 2  # 100, 100
NTAP = 3 * KH * KW  # 75
N_CORES = 8
NP = 4  # batches per core
NF = F // 2  # filters per core
NCHUNK = 3  # h chunks of 32 rows
CB = 32  # rows per channel block

CFG_DTYPE = os.environ.get("KCFG_DTYPE", "fp16")
CFG_ACT16 = int(os.environ.get("KCFG_ACT16", "12"))  # ACT sub cols per 16
CFG_GPTAPS = int(os.environ.get("KCFG_GPTAPS", "0"))  # whole taps on gpsimd
CFG_LAG = int(os.environ.get("KCFG_LAG", "7"))  # taps of sub->min pipelining
CFG_RTMP = int(os.environ.get("KCFG_RTMP", "10"))  # tmp ring depth (> LAG+1)
CFG_BIGMIN = int(os.environ.get("KCFG_BIGMIN", "0"))  # 1: one FD=4608 min/tap
CFG_PAIRMIN = int(os.environ.get("KCFG_PAIRMIN", "1"))  # 1: two FD=2304 mins/tap
CFG_ABSK = int(os.environ.get("KCFG_ABSK", "2"))  # ACT absorber every k-th ACT tap
CFG_REPEAT = int(os.environ.get("KCFG_REPEAT", "1"))

_DT = {
    "fp32": (mybir.dt.float32, np.float32, 1e30),
    "fp16": (mybir.dt.float16, np.float16, 30000.0),
    "bf16": (mybir.dt.bfloat16, None, 1e30),
}

_prog_cache = {}
LAST_RESULTS = None


def _np_dtype(name):
    if name == "bf16":
        import ml_dtypes

        return np.dtype(ml_dtypes.bfloat16)
    return np.dtype(_DT[name][1])


def _build_program(dtype_name, act16, gptaps, repeat=1):
    # The kernel-tail Drain must wait on every sem lane used; cap the SWDGE
    # completion-sem lanes so it fits the CTRL struct's sync-wait capacity.
    import concourse.tile_sem_assignment as _tsa

    _orig_swdge = _tsa.NUM_SWDGE_GLOBAL_SEMS
    _tsa.NUM_SWDGE_GLOBAL_SEMS = 4
    try:
        return _build_program_inner(dtype_name, act16, gptaps, repeat)
    finally:
        _tsa.NUM_SWDGE_GLOBAL_SEMS = _orig_swdge


class _SplitDrainTC(tile.TileContext):
    """TileContext whose kernel-tail drain is split into one drain per sem
    lane: the stock single Drain carries a wait for every lane used, which
    overflows the CTRL struct's sync-wait encoding on this compiler."""

    def _drain_and_barrier(self, tick_clock, wait_clock):
        from concourse.tile_sem_assignment import N_PROCS
        from concourse.vector_clock import ScopedClock, VectorClock

        gc = tick_clock.global_clock
        ticks = [gc[p] for p in range(N_PROCS)]
        for p in range(N_PROCS):
            if ticks[p] <= 0:
                continue
            sub = [ticks[q] if q == p else 0 for q in range(N_PROCS)]
            d = self.nc.sync.drain()
            wait_clock.add_sem_waits(d.ins, ScopedClock({None: VectorClock(sub)}))

        self.nc.all_engine_barrier()
        assert self.sems is not None
        popped = self.nc._tile_sem_poison_stack.pop()
        assert popped is self._sem_poison
        self.nc.clear_and_free_semaphores(list(self.sems.allocated().values()))
        self.nc.all_engine_barrier()


def _tap_schedule(act16, gptaps):
    """Returns (gp_set, sub_eng) where sub_eng[(ti, f)] in {"dve", "act"}
    for non-gp taps.  Tap 0 is always fully on DVE (direct acc init).

    Odd-dx taps go to ACT first (ACT needs no 4B alignment, so if every
    odd-dx subtract lands on ACT the shifted in_b tiles are never read by
    DVE and are dropped entirely); the remaining ACT quota spreads over the
    even-dx taps via Bresenham."""
    assert 0 <= gptaps <= 40
    gp_set = set()
    if gptaps:
        idxs = np.linspace(1, NTAP - 1, gptaps).round().astype(int)
        gp_set = set(int(i) for i in idxs)
        assert len(gp_set) == gptaps
    taps = [(k, dy, dx) for k in range(3) for dy in range(KH) for dx in range(KW)]
    elig = [ti for ti in range(1, NTAP) if ti not in gp_set]
    odd = [ti for ti in elig if taps[ti][2] % 2 == 1]
    even = [ti for ti in elig if taps[ti][2] % 2 == 0]
    a_target = (len(elig) * 4 * act16) // 16
    sub_eng = {(0, f): "dve" for f in range(NF)}
    # odd taps first, whole taps at a time, spread evenly through the rep
    n_odd_act = min(a_target // 4, len(odd))
    odd_act = set()
    for q2, ti in enumerate(odd):
        if ((q2 + 1) * n_odd_act) // len(odd) - (q2 * n_odd_act) // len(odd) == 1:
            odd_act.add(ti)
    for ti in odd:
        for f in range(NF):
            sub_eng[ti, f] = "act" if ti in odd_act else "dve"
    rem = a_target - 4 * n_odd_act
    ncols = len(even) * 4
    q = 0
    for ti in even:
        for f in range(NF):
            on_act = rem > 0 and ((q + 1) * rem) // ncols - (q * rem) // ncols == 1
            sub_eng[ti, f] = "act" if on_act else "dve"
            q += 1
    return gp_set, sub_eng


def _build_program_inner(dtype_name, act16, gptaps, repeat=1):
    dt, _, _ = _DT[dtype_name]
    nc = bass.Bass()
    # Input planes: [k, c, h_pad, pair, w_pad]; xin_b is xin shifted one w
    # element so odd-dx windows stay 4B-aligned for the DVE packed modes.
    xin = nc.declare_dram_parameter("xin", [3, C, HP, NP, WP], dt, isOutput=False)
    xin_b = nc.declare_dram_parameter("xin_b", [3, C, HP, NP, WP], dt, isOutput=False)
    nkk = NTAP * NF
    kkin = nc.declare_dram_parameter("kk", [128, 2 * nkk], mybir.dt.float32, isOutput=False)
    yout = nc.declare_dram_parameter("yout", [128, NF, NCHUNK, NP, W], dt, isOutput=True)

    gp_set, sub_eng = _tap_schedule(act16, gptaps)

    with _SplitDrainTC(nc) as tc, ExitStack() as ctx:
        pool = ctx.enter_context(tc.tile_pool(name="main", bufs=1))

        # Compute-instruction ISA slots can encode only ONE sync wait, so
        # "touch" every DMA'd region with a trivial op on each consuming
        # engine right after its DMA; later compute instructions inherit the
        # dependency through engine program order.
        touch_v = pool.tile([1, 4096], mybir.dt.float32, name="touch_v", tag="touch_v")
        touch_s = pool.tile([1, 4096], mybir.dt.float32, name="touch_s", tag="touch_s")
        touch_g = pool.tile([1, 4096], mybir.dt.float32, name="touch_g", tag="touch_g")
        tctr = [0, 0, 0]

        def _touch(src):
            i = tctr[0] = tctr[0] + 1
            nc.vector.tensor_scalar_add(touch_v[0:1, i : i + 1], src, 0.0)
            i = tctr[1] = tctr[1] + 1
            nc.scalar.copy(touch_s[0:1, i : i + 1], src)
            i = tctr[2] = tctr[2] + 1
            nc.gpsimd.tensor_scalar_add(touch_g[0:1, i : i + 1], src, 0.0)

        # One HWDGE dma_start fans out over several HW queues; the software
        # DGE (gpsimd engine) uses a single queue -> single completion sem.
        dma = nc.gpsimd.dma_start

        taps = [(k, dy, dx) for k in range(3) for dy in range(KH) for dx in range(KW)]
        # in_b (the w-shifted copy for 4B alignment) is only needed by DVE
        # subtracts on odd-dx taps; ACT and gpsimd read in_a unaligned.
        used_b = set()
        for ti, (k, dy, dx) in enumerate(taps):
            if dx % 2 == 1 and ti not in gp_set and any(
                sub_eng[ti, f] == "dve" for f in range(NF)
            ):
                used_b.add((k, dy))

        # dy-shifted input tiles, one per (k, dy): [p = 32c+j, m, np, wp]
        # holding xin[k, c, 32m+j+dy, np, wp].
        in_a = {}
        in_b = {}
        for k in range(3):
            for dy in range(KH):
                pairs = [(xin, in_a, "a")]
                if (k, dy) in used_b:
                    pairs.append((xin_b, in_b, "b"))
                for src_dram, dst_map, nm in pairs:
                    t = pool.tile([128, NCHUNK, NP, WP], dt,
                                  name=f"in{nm}_{k}_{dy}", tag=f"in{nm}_{k}_{dy}")
                    for m in range(NCHUNK):
                        for c in range(C):
                            dma(t[c * CB : (c + 1) * CB, m],
                                src_dram[k, c, CB * m + dy : CB * m + dy + CB])
                            _touch(t[c * CB : c * CB + 1, m, 0, 0:1])
                    dst_map[k, dy] = t

        kkt = pool.tile([128, 2 * nkk], mybir.dt.float32, name="kkt", tag="kkt")
        dma(kkt[:], kkin[:])
        _touch(kkt[0:1, 0:1])

        acc = pool.tile([128, NF, NCHUNK, NP, W], dt, name="acc", tag="acc")
        acc_gp = None
        tmp_g = []
        if gptaps:
            acc_gp = pool.tile([128, NF, NCHUNK, NP, W], dt, name="acc_gp", tag="acc_gp")
            tmp_g = [pool.tile([128, NCHUNK, NP, W], dt, name=f"tmpg_{i}", tag=f"tmpg_{i}")
                     for i in range(2)]
        any_act = any(v == "act" for v in sub_eng.values())
        bigmin = CFG_BIGMIN and not any_act
        if bigmin:
            # All-DVE: one contiguous ring tile per slot so a single FD=4608
            # tensor_tensor min covers all four filters (same-engine waits
            # collapse to one).
            ringb = [pool.tile([128, NF, NCHUNK, NP, W], dt, name=f"ringb_{i}",
                               tag=f"ringb_{i}") for i in range(CFG_RTMP)]
            ring = [[ringb[i][:, f] for f in range(NF)] for i in range(CFG_RTMP)]
        elif CFG_PAIRMIN:
            # Filter pairs share a contiguous tile so one FD=2304 min covers
            # two filters (halves min instruction overhead; acc chain
            # distance 2).  Cross-engine waits stay <= 1 because each ACT
            # producer still gets its own absorber touch.
            ringp = [[pool.tile([128, 2, NCHUNK, NP, W], dt, name=f"ringp_{i}_{p}",
                                tag=f"ringp_{i}_{p}") for p in range(NF // 2)]
                     for i in range(CFG_RTMP)]
            ring = [[ringp[i][f // 2][:, f % 2] for f in range(NF)]
                    for i in range(CFG_RTMP)]
        else:
            # One tile per (ring slot, filter): every tmp tile has exactly ONE
            # writer instruction, so each consuming min carries exactly one
            # cross-engine sem wait (compute ISA slots can encode only one).
            ring = [[pool.tile([128, NCHUNK, NP, W], dt, name=f"ring_{i}_{f}",
                               tag=f"ring_{i}_{f}") for f in range(NF)]
                    for i in range(CFG_RTMP)]

        def win_of(ti, aligned):
            """Window AP for tap ti; aligned=True returns the 4B-aligned view
            (via in_b for odd dx) for DVE packed modes."""
            k, dy, dx = taps[ti]
            use_b = aligned and dx % 2 == 1
            dxa = dx - 1 if use_b else dx
            src = (in_b if use_b else in_a)[k, dy]
            return src[:, :, :, dxa : dxa + W]

        # Tile emits a sem wait per operand-producing engine tick, including
        # same-engine ones, and a compute instruction can encode only ONE.
        # Two absorber touches per tap keep every hot instruction at <= 1:
        #   - a DVE touch reads the tap's last ACT-written ring tile (1 ACT
        #     wait); the following mins then carry only their acc-chain DVE
        #     wait.  Its touch_v column doubles as a progress marker whose
        #     tick precedes this tap's mins.
        #   - an ACT copy reads a recent progress marker (1 DVE wait); the
        #     following ACT ring rewrites then carry only their same-engine
        #     WAW wait.
        marker_cols = []
        slot_box = [0]
        ring_pos = [0]
        last_abs = [-10**9]
        pending = []  # (ti, ring slot) awaiting their DVE min pass
        if CFG_ABSK > 1:
            assert CFG_RTMP >= CFG_LAG + 2 + (CFG_ABSK - 1)

        for _rep in range(repeat):
            first_gp = True

            def flush_one():
                ti, s = pending.pop(0)
                has_act = [f for f in range(NF) if sub_eng[ti, f] == "act"]
                # Sync bookkeeping is per producing instruction, so one
                # absorber per ACT-written tile (each carries that single ACT
                # wait); the mins then carry only their same-engine acc wait.
                for f in has_act:
                    i = tctr[0] = tctr[0] + 1
                    nc.vector.tensor_scalar_add(touch_v[0:1, i : i + 1],
                                                ring[s][f][0:1, 0, 0, 0:1], 0.0)
                if any_act:
                    if not has_act:
                        i = tctr[0] = tctr[0] + 1
                        nc.vector.tensor_scalar_add(touch_v[0:1, i : i + 1],
                                                    acc[0:1, 0, 0, 0, 0:1], 0.0)
                    marker_cols.append(i)
                if bigmin:
                    nc.vector.tensor_tensor(acc[:], ringb[s][:], acc[:],
                                            mybir.AluOpType.min)
                elif CFG_PAIRMIN:
                    for p in range(NF // 2):
                        nc.vector.tensor_tensor(acc[:, 2 * p : 2 * p + 2],
                                                ringp[s][p][:],
                                                acc[:, 2 * p : 2 * p + 2],
                                                mybir.AluOpType.min)
                else:
                    for f in range(NF):
                        nc.vector.tensor_tensor(acc[:, f], ring[s][f][:], acc[:, f],
                                                mybir.AluOpType.min)

            for ti in range(NTAP):
                if ti in gp_set:
                    wsel = win_of(ti, False)
                    if first_gp:
                        for f in range(NF):
                            kk_ap = kkt[:, ti * NF + f : ti * NF + f + 1]
                            nc.gpsimd.tensor_scalar(acc_gp[:, f], wsel, kk_ap, None,
                                                    mybir.AluOpType.subtract)
                        first_gp = False
                    else:
                        for f in range(NF):
                            kk_ap = kkt[:, ti * NF + f : ti * NF + f + 1]
                            tg = tmp_g[f % 2]
                            nc.gpsimd.tensor_scalar(tg[:], wsel, kk_ap, None,
                                                    mybir.AluOpType.subtract)
                            nc.gpsimd.tensor_tensor(acc_gp[:, f], tg[:], acc_gp[:, f],
                                                    mybir.AluOpType.min)
                    continue
                if ti == 0:
                    wsel = win_of(ti, True)
                    for f in range(NF):
                        kk_ap = kkt[:, ti * NF + f : ti * NF + f + 1]
                        nc.vector.tensor_scalar(acc[:, f], wsel, kk_ap, None,
                                                mybir.AluOpType.subtract)
                    continue
                s = slot_box[0]
                slot_box[0] = (s + 1) % CFG_RTMP
                ring_pos[0] += 1
                # ACT subs first so the scalar engine runs ahead
                if any_act and marker_cols and any(sub_eng[ti, f] == "act" for f in range(NF)):
                    if ring_pos[0] - last_abs[0] >= CFG_ABSK:
                        mc = marker_cols[-1]
                        i = tctr[1] = tctr[1] + 1
                        nc.scalar.copy(touch_s[0:1, i : i + 1], touch_v[0:1, mc : mc + 1])
                        last_abs[0] = ring_pos[0]
                for f in range(NF):
                    if sub_eng[ti, f] == "act":
                        negkk_ap = kkt[:, nkk + ti * NF + f : nkk + ti * NF + f + 1]
                        nc.scalar.activation(ring[s][f][:], win_of(ti, False),
                                             mybir.ActivationFunctionType.Identity,
                                             bias=negkk_ap)
                for f in range(NF):
                    if sub_eng[ti, f] == "dve":
                        kk_ap = kkt[:, ti * NF + f : ti * NF + f + 1]
                        nc.vector.tensor_scalar(ring[s][f][:], win_of(ti, True),
                                                kk_ap, None, mybir.AluOpType.subtract)
                pending.append((ti, s))
                if len(pending) > CFG_LAG:
                    flush_one()
            # pending mins carry across the rep boundary: min is idempotent,
            # so the stale tmps folded into the next rep's acc change nothing.
            if gptaps:
                while pending:
                    flush_one()
                for f in range(NF):
                    nc.vector.tensor_tensor(acc[:, f], acc_gp[:, f], acc[:, f],
                                            mybir.AluOpType.min)

        while pending:
            flush_one()

        # Pool-engine touch absorbs the DVE dependency (1 wait) so the SWDGE
        # out-DMA needs only its queue-FIFO wait.
        i = tctr[2] = tctr[2] + 1
        nc.gpsimd.tensor_scalar_add(touch_g[0:1, i : i + 1], acc[0:1, 0, 0, 0, 0:1], 0.0)
        dma(yout[:], acc[:])

    return nc


def _get_program(dtype_name, act16, gptaps, repeat=1):
    key = (dtype_name, act16, gptaps, repeat, CFG_LAG, CFG_RTMP, CFG_BIGMIN, CFG_PAIRMIN, CFG_ABSK)
    if key not in _prog_cache:
        _prog_cache[key] = _build_program(dtype_name, act16, gptaps, repeat)
    return _prog_cache[key]


def _krev(kernel):
    """[g, dy, dx, k, c, f] rotated/reversed SE, pure re-indexing of `kernel`."""
    k_ero = np.stack(
        [
            np.rot90(kernel[:, :, 2], k=3, axes=(0, 1)),
            kernel[:, :, 1],
            np.rot90(kernel[:, :, 0], k=1, axes=(0, 1)),
        ],
        axis=2,
    )
    krot = np.stack([np.rot90(k_ero, k=j, axes=(0, 1)) for j in range(4)], axis=0)
    return krot[:, ::-1, ::-1]


def _core_units(core):
    g = core // 2
    fh = core % 2
    return g, list(range(B)), list(range(fh * NF, fh * NF + NF))


def _make_in_map(x, kr, core, np_dt, big):
    g, bs, fs = _core_units(core)
    planes = np.full((3, C, HP, NP, WP), big, np.float32)
    for pi, b in enumerate(bs):
        for k in range(3):
            src = x[b, (g + k - 1) % 4]  # [H, W, C]
            planes[k, :, PAD : PAD + H, pi, PAD : PAD + W] = src.transpose(2, 0, 1)
    planes_b = np.full_like(planes, big)
    planes_b[..., : WP - 1] = planes[..., 1:]
    sel = kr[g][:, :, :, :, fs]  # [dy, dx, k, c, NF]
    taps_kcf = np.ascontiguousarray(sel.transpose(2, 0, 1, 3, 4))  # [k,dy,dx,c,NF]
    tap_cf = taps_kcf.reshape(NTAP, C, NF)
    cs = np.repeat(np.arange(C), CB)  # channel per partition
    kkcols = tap_cf[:, cs, :].transpose(1, 0, 2).reshape(128, NTAP * NF)
    kk = np.concatenate([kkcols, -kkcols], axis=1)
    return {
        "xin": planes.astype(np_dt),
        "xin_b": planes_b.astype(np_dt),
        "kk": np.ascontiguousarray(kk.astype(np.float32)),
    }


def _assemble(results):
    out = np.zeros((B, G, H, W, F), np.float32)
    for core in range(N_CORES):
        g, bs, fs = _core_units(core)
        y = np.asarray(results[core]["yout"]).astype(np.float32)  # [128,NF,NCHUNK,NP,W]
        s = y.reshape(C, CB, NF, NCHUNK, NP, W).sum(axis=0)  # [CB, NF, NCHUNK, NP, W]
        hform = s.transpose(3, 1, 2, 0, 4).reshape(NP, NF, H, W)
        for pi, b in enumerate(bs):
            out[b, g, :, :, fs[0] : fs[0] + NF] = hform[pi].transpose(1, 2, 0)
    return out


def kernel(x, kernel):
    x = np.ascontiguousarray(np.asarray(x, dtype=np.float32))
    se = np.ascontiguousarray(np.asarray(kernel, dtype=np.float32))
    np_dt = _np_dtype(CFG_DTYPE)
    big = _DT[CFG_DTYPE][2]

    kr = _krev(se)  # [g, dy, dx, k, c, f]
    in_maps = [_make_in_map(x, kr, core, np_dt, big) for core in range(N_CORES)]

    nc = _get_program(CFG_DTYPE, CFG_ACT16, CFG_GPTAPS, CFG_REPEAT)
    res = run_bass_kernel_spmd(nc, in_maps, list(range(N_CORES)), trace=False)
    global LAST_RESULTS
    LAST_RESULTS = res
    return _assemble(res.results)
